# revision 54
# baseline (speedup 1.0000x reference)
"""Trainium2 Bass kernel for nn_LossSupervisedTags (tag + heatmap MSE loss).

Contract: kernel(**inputs) takes the FULL unsharded inputs (as produced by
setup_inputs) and returns the FULL scalar output.  Internally the batch dim
(B=32) is sharded 4-images-per-core across 8 NeuronCores; each core computes
its local tag / heatmap loss partial sums on device, and the host combines
the 8 partial sums into the final scalar mean.

Host staging: slices per-core shards, transposes dets/heat to [h, p, w] so
every DMA descriptor is a 4.3KB contiguous run, and gathers the 510 predicted
tags per image (index staging for the tag loss).

Per-core device pipeline (per image b, stacks s=0..3):
  DVE   : diff = dets[b,s] - heat[b]      (layout [h=128, (p,w)=2176], bf16 2x)
  ACT/DVE: sq  = diff^2 into sq_img[:, s*2176:...] as bf16 (ACT Square runs
          1x, DVE bf16 mul 2x; one DVE square per image balances the engines
          against the DMA pace)
  PE    : 32 matmuls per image, each covering a group of 4 w-columns:
          lhsT = mask[:, 4g:4g+4] (bf16), rhs = sq viewed as [h, w4, s, p].
          Group g targets PE column quadrant g%4 (tile_position) so
          consecutive LDWEIGHTS+MATMUL pairs pipeline in different
          sub-arrays.  One PSUM region accumulates across ALL images
          (the loss sums over b anyway): useful results sit at
          psum[32q+j, 68j:68j+68] = sum over {w : w%16 == 4q+j} of m * sq.
  tag   : gathered pred tags packed [128,64]; (pt-gt)^2*vis summed on DVE in
          an early bubble (input arrives on the scalar HWDGE ring).

DMA plan: 20 single [128, 2176]-shaped transfers on the sync HWDGE ring in
need-order, one tile per transfer (multiple outstanding DMAs into one tile
serialize on its semaphore).  mask/tagin and the tag output ride the scalar
HWDGE ring; descriptors within a ring drain FIFO.
"""

import sys
import types

import ml_dtypes
import numpy as np

import concourse.bacc as bacc
import concourse.mybir as mybir
from concourse.tile import TileContext
from concourse.bass_utils import run_bass_kernel_spmd

# If BASS_TRACE is set in the environment but this image lacks
# antenv.axon_hooks, run_bass_kernel_spmd would die on import; register a
# no-op hook module so tracing degrades gracefully instead.
try:
    import antenv.axon_hooks  # noqa: F401
except ImportError:
    try:
        import antenv

        _m = types.ModuleType("antenv.axon_hooks")
        _m.get_axon_ntff_profile_hook = lambda: None
        _m.set_axon_ntff_profile_hook = lambda h: None
        sys.modules["antenv.axon_hooks"] = _m
        antenv.axon_hooks = _m
    except ImportError:
        pass

# Problem constants (hardcoded per harness contract)
B, S, C, H, W = 32, 4, 34, 128, 128
N_PARTS, TAG_DIM, M = 17, 1, 30
TAG_W, HM_W = 0.001, 1.0
NCORES = 8
BLOC = B // NCORES            # 4 images per core
FREE = N_PARTS * W            # 2176 free elems per (b, s) tile
KP = M * N_PARTS              # 510 keypoints per image
KP_COLS = 4                   # ceil(510 / 128) columns per (b, s)
TAG_COLS = BLOC * S * KP_COLS  # 64
WG = 4                        # w-columns per matmul group
NG = W // WG                  # 32 matmul groups per image
SQF = WG * S * N_PARTS        # 272 psum cols per image (w4, s, p)

_cache = {}

# DMA dets from HBM as fp8e4m3 and upcast to bf16 in-flight (SWDGE cast
# path).  Measured SLOWER than bf16 HWDGE: the stream is SBUF-write-side
# bound (11.1MB bf16 lands in SBUF either way) and the cast path costs more
# DMA-engine time per byte (28us vs 26us busy), so keep plain bf16.
FP8_DETS = False


def _build():
    f32 = mybir.dt.float32
    bf16 = mybir.dt.bfloat16
    det_dt = mybir.dt.float8e4 if FP8_DETS else mybir.dt.bfloat16
    nc = bacc.Bacc(
        "TRN2", target_bir_lowering=False, debug=False, num_devices=NCORES
    )
    # Host pre-interleaves dets into 2-stack pairs [pair, H, (s p w)] and
    # heat into 2-image pairs [pair, H, (b p w)] so every bulk transfer is a
    # single contiguous 8.7KB run per partition (max DMA-engine efficiency);
    # image 3's last two stacks stay as single-stack transfers [H, (p w)]
    # for fine-grained tail chasing.
    dets_p = nc.dram_tensor(
        "dets_p", [7, H, 2 * FREE], det_dt, kind="ExternalInput"
    )
    dets_s = nc.dram_tensor("dets_s", [2, H, FREE], det_dt, kind="ExternalInput")
    heat_p = nc.dram_tensor("heat_p", [2, H, 2 * FREE], bf16, kind="ExternalInput")
    maskw = nc.dram_tensor("maskw", [H, BLOC * W], bf16, kind="ExternalInput")
    tagin = nc.dram_tensor("tagin", [128, 3 * TAG_COLS], f32, kind="ExternalInput")
    out_det = nc.dram_tensor("out_det", [128, SQF], f32, kind="ExternalOutput")
    out_tag = nc.dram_tensor("out_tag", [128, 1], f32, kind="ExternalOutput")

    with TileContext(nc) as tc:
        with (
            tc.tile_pool(name="const", bufs=1) as cpool,
            tc.tile_pool(name="heatp", bufs=2) as hpool,
            tc.tile_pool(name="detp", bufs=7) as dpool,
            tc.tile_pool(name="dets3", bufs=2) as spool,
            tc.tile_pool(name="diffp", bufs=4) as fpool,
            tc.tile_pool(name="sqp", bufs=3) as qpool,
            tc.tile_pool(name="psum", bufs=1, space="PSUM") as ppool,
        ):
            # Small inputs ride the scalar HWDGE ring so they don't queue
            # behind the bulk det stream on the sync ring.
            mask_t = cpool.tile([128, BLOC * W], bf16)
            nc.scalar.dma_start(out=mask_t[:], in_=maskw[:])
            tag_t = cpool.tile([128, 3 * TAG_COLS], f32)
            nc.scalar.dma_start(out=tag_t[:], in_=tagin[:])

            # Dummy activation so the Square table set loads during the DMA
            # ramp instead of delaying the first real square.
            warm_t = cpool.tile([1, 8], f32)
            warm_o = cpool.tile([1, 8], f32)
            nc.gpsimd.memset(warm_t[:], 0.0)
            nc.scalar.activation(
                warm_o[:], warm_t[:], mybir.ActivationFunctionType.Square
            )

            # Bulk het/det stream on the sync ring: 20 single transfers, each
            # into its OWN tile.  Measured facts behind this shape: (a) the
            # [128, 2176] single-stack transfer with one 4.3KB run per
            # partition is the most DMA-engine-efficient unit (pairing
            # stacks into [128, 2, 2176] transfers measured ~10% slower per
            # byte); (b) multiple outstanding DMAs into ONE tile serialize
            # on that tile's semaphore, so every transfer gets its own tile;
            # (c) the dynamic HWDGE path keeps ~10 transfers in flight and
            # stalls the issuing sequencer until ~28us when full — with this
            # order the stalled issues are images 2-3, which still arrive
            # before they're needed.  Within the ring, descriptors drain
            # FIFO (emission order = arrival order).
            # 11 transfers up-front on the sync ring in need-order, one tile
            # per transfer.  Measured facts behind this shape: (a) single
            # contiguous runs per partition are the efficient DMA unit (the
            # host interleave makes pairs one 8.7KB run); (b) multiple
            # outstanding DMAs into ONE tile serialize on that tile's
            # semaphore; (c) the dynamic-HWDGE path keeps ~10 transfers in
            # flight and stalls the issuing sequencer until ~28us when full
            # — with 11 transfers only the last (needed ~41us) stalls, and
            # its descriptors still enter the ring in time.  Within a ring,
            # descriptors drain FIFO (emission order = arrival order).
            heat_tiles = {}
            det_tiles = {}
            pair_idx = 0
            for b0 in (0, 2):
                hp = hpool.tile([128, 2 * FREE], bf16, name="heat_t", tag="heat_t")
                nc.sync.dma_start(out=hp[:], in_=heat_p[b0 // 2])
                heat_tiles[b0] = hp[:, 0:FREE]
                heat_tiles[b0 + 1] = hp[:, FREE : 2 * FREE]
                for b in (b0, b0 + 1):
                    for s0 in (0, 2):
                        if b == BLOC - 1 and s0 == 2:
                            for s in (2, 3):
                                ds = spool.tile(
                                    [128, FREE], bf16, name="det_s", tag="det_s"
                                )
                                nc.sync.dma_start(out=ds[:], in_=dets_s[s - 2])
                                det_tiles[(b, s)] = ds[:]
                        else:
                            dp = dpool.tile(
                                [128, 2 * FREE], bf16, name="det_t", tag="det_t"
                            )
                            nc.sync.dma_start(out=dp[:], in_=dets_p[pair_idx])
                            pair_idx += 1
                            det_tiles[(b, s0)] = dp[:, 0:FREE]
                            det_tiles[(b, s0 + 1)] = dp[:, FREE : 2 * FREE]

            acc_det = cpool.tile([128, SQF], f32)
            # One PSUM region accumulates across ALL images (the final loss
            # sums over b anyway), so there is a single PSUM->SBUF copy and
            # a tiny output DMA at the very end.
            psum_t = ppool.tile([128, SQF], f32)
            psum_r = psum_t[:].rearrange("m (j s p) -> m j s p", j=WG, s=S)

            # ---- heatmap (det) loss ----
            for b in range(BLOC):
                if b == 1:
                    # Tag loss (tiny).  Emitted here so it sits behind image
                    # 0's subs in the DVE program: the first det sub is the
                    # critical op at pipeline start, the tag block fills a
                    # later DVE bubble.
                    ptg_t = tag_t[:, 0:TAG_COLS]
                    gtv_t = tag_t[:, TAG_COLS : 2 * TAG_COLS]
                    vis_t = tag_t[:, 2 * TAG_COLS : 3 * TAG_COLS]
                    e_t = cpool.tile([128, TAG_COLS], f32)
                    ev_t = cpool.tile([128, TAG_COLS], f32)
                    scr_t = cpool.tile([128, TAG_COLS], f32)
                    tag_acc = cpool.tile([128, 1], f32)
                    nc.vector.tensor_sub(e_t[:], ptg_t, gtv_t)
                    nc.vector.tensor_mul(ev_t[:], e_t[:], vis_t)
                    nc.vector.tensor_mul(scr_t[:], e_t[:], ev_t[:])
                    nc.vector.reduce_sum(
                        tag_acc[:], scr_t[:], axis=mybir.AxisListType.X
                    )
                    nc.scalar.dma_start(out=out_tag[:], in_=tag_acc[:])

                heat_t = heat_tiles[b]
                sq_t = qpool.tile([128, S * FREE], bf16)
                for s in range(S):
                    det_t = det_tiles[(b, s)]
                    diff_t = fpool.tile([128, FREE], bf16, name="diff_t", tag="diff_t")
                    dst = sq_t[:, s * FREE : (s + 1) * FREE]
                    # ACT squares run at 1x (2.1us); DVE bf16 muls at 2x
                    # (1.2us).  One DVE square per image balances the
                    # engines against the DMA pace; the final tile is
                    # processed in halves, fully on DVE, to shorten the tail.
                    nc.vector.tensor_sub(diff_t[:], det_t, heat_t)
                    # DVE paces the tail: keep its square load on the late
                    # images only (ACT has mid-stream slack for the rest).
                    if (s == 1 and b >= 2) or (b == BLOC - 1 and s == S - 1):
                        nc.vector.tensor_mul(dst, diff_t[:], diff_t[:])
                    else:
                        nc.scalar.activation(
                            dst, diff_t[:], mybir.ActivationFunctionType.Square
                        )

                # 32 matmuls per image, each covering 4 w-columns: lhsT =
                # mask 4-col slice, rhs = sq viewed [h, w4, s, p].  Group g
                # targets PE column quadrant g % 4 so consecutive
                # LDWEIGHTS+MATMUL pairs run concurrently in different
                # sub-arrays instead of serializing on the array-column
                # conflict.  Useful outputs are psum[32q+j, 68j:68(j+1)].
                # The last image's data all arrives at the stream's end, so
                # its rounds can't overlap the stream; two rounds ({s0,s1}
                # runs while DVE finishes s2/s3, then {s2,s3}) minimize
                # round overhead + inter-round semaphore waits in the tail.
                sq_g = sq_t[:].rearrange("q (s p w) -> q w s p", s=S, p=N_PARTS)
                rounds = [(0, S)] if b < BLOC - 1 else [(0, 2), (2, 4)]
                last_round = len(rounds) - 1
                for ri, (s0, s1) in enumerate(rounds):
                    for g in range(NG):
                        q = g % 4
                        nc.tensor.matmul(
                            psum_r[32 * q : 32 * q + 4, :, s0:s1, :],
                            lhsT=mask_t[:, b * W + WG * g : b * W + WG * (g + 1)],
                            rhs=sq_g[:, WG * g : WG * (g + 1), s0:s1, :],
                            start=(b == 0 and g < 4),
                            stop=(b == BLOC - 1 and ri == last_round and g >= NG - 4),
                            tile_position=(0, 32 * q),
                        )

            # Output DMA issues from the ACT engine itself (scalar ring) so
            # the copy->DMA handoff needs no cross-engine semaphore hop.
            nc.scalar.copy(acc_det[:], psum_t[:])
            nc.scalar.dma_start(out=out_det[:], in_=acc_det[:])
    nc.compile()
    return nc


def _pack(vals):
    """vals: (BLOC, S, KP) float32 -> [128, TAG_COLS] with col = b*16+s*4+j,
    partition k holding element j*128+k of the zero-padded 512 vector."""
    padded = np.zeros((BLOC, S, KP_COLS * 128), np.float32)
    padded[..., :KP] = vals
    return (
        padded.reshape(BLOC, S, KP_COLS, 128)
        .transpose(3, 0, 1, 2)
        .reshape(128, TAG_COLS)
    )


def kernel(preds, masks, keypoints_idx, keypoints_vis, gt_tags, heatmaps):
    preds = np.asarray(preds, dtype=np.float32)
    masks = np.asarray(masks, dtype=np.float32)
    keypoints_idx = np.asarray(keypoints_idx)
    keypoints_vis = np.asarray(keypoints_vis, dtype=np.float32)
    gt_tags = np.asarray(gt_tags, dtype=np.float32)
    heatmaps = np.asarray(heatmaps, dtype=np.float32)

    if "nc" not in _cache:
        _cache["nc"] = _build()
    nc = _cache["nc"]

    # Host-side input staging: gather predicted tags at keypoint locations
    # (index-based staging; all loss arithmetic runs on device).
    tags = preds[:, :, N_PARTS:].reshape(B, S, N_PARTS * H * W)
    flat_idx = keypoints_idx.reshape(B, 1, KP).astype(np.int64)
    pt = np.take_along_axis(tags, np.broadcast_to(flat_idx, (B, S, KP)), axis=2)
    gt = gt_tags.reshape(B, KP)
    vi = keypoints_vis.reshape(B, KP)

    in_maps = []
    for c in range(NCORES):
        b0 = c * BLOC
        sl = slice(b0, b0 + BLOC)
        tag_in = np.concatenate(
            [
                _pack(pt[sl]),
                _pack(np.broadcast_to(gt[sl][:, None, :], (BLOC, S, KP))),
                _pack(np.broadcast_to(vi[sl][:, None, :], (BLOC, S, KP))),
            ],
            axis=1,
        )
        det_np = ml_dtypes.float8_e4m3 if FP8_DETS else ml_dtypes.bfloat16
        # [BLOC, S, 17, H, W] -> [BLOC, S, H, 17, W], then pack 2-stack
        # pairs as [H, (s p w)] and heat 2-image pairs as [H, (b p w)] so
        # each bulk DMA is one contiguous run per partition.
        dt_ = preds[sl, :, :N_PARTS].transpose(0, 1, 3, 2, 4)
        ht_ = heatmaps[sl].transpose(0, 2, 1, 3)
        dets_p = np.stack(
            [
                dt_[b, s0 : s0 + 2].transpose(1, 0, 2, 3).reshape(H, 2 * FREE)
                for b, s0 in (
                    (0, 0), (0, 2), (1, 0), (1, 2), (2, 0), (2, 2), (3, 0),
                )
            ]
        )
        dets_s = dt_[3, 2:4].reshape(2, H, FREE)
        heat_pk = np.stack(
            [
                ht_[b0 : b0 + 2].transpose(1, 0, 2, 3).reshape(H, 2 * FREE)
                for b0 in (0, 2)
            ]
        )
        in_maps.append(
            {
                "dets_p": np.ascontiguousarray(dets_p).astype(det_np),
                "dets_s": np.ascontiguousarray(dets_s).astype(det_np),
                "heat_p": np.ascontiguousarray(heat_pk).astype(ml_dtypes.bfloat16),
                # [BLOC, H, W] -> [H, BLOC*W]
                "maskw": np.ascontiguousarray(
                    masks[sl].transpose(1, 0, 2).reshape(H, BLOC * W)
                ).astype(ml_dtypes.bfloat16),
                "tagin": np.ascontiguousarray(tag_in),
            }
        )

    res = run_bass_kernel_spmd(nc, in_maps, list(range(NCORES)))
    _cache["last_results"] = res

    det_total = 0.0
    tag_total = 0.0
    for r in res.results:
        od = r["out_det"].astype(np.float64).reshape(4, 32, WG, S * N_PARTS)
        # row 32q+i, diag block i holds the w % 16 == 4q+i partials
        for q in range(4):
            for i in range(WG):
                det_total += float(od[q, i, i, :].sum())
        tag_total += float(r["out_tag"].sum(dtype=np.float64))

    det_mean = det_total / (B * S * N_PARTS * H * W)
    tag_mean = tag_total / (B * S)
    return np.float32(TAG_W * tag_mean + HM_W * det_mean)


# revision 55
# speedup vs baseline: 1.0455x; 1.0455x over previous
"""Trainium2 Bass kernel for nn_LossSupervisedTags (tag + heatmap MSE loss).

Contract: kernel(**inputs) takes the FULL unsharded inputs (as produced by
setup_inputs) and returns the FULL scalar output.  Internally the batch dim
(B=32) is sharded 4-images-per-core across 8 NeuronCores; each core computes
its local tag / heatmap loss partial sums on device, and the host combines
the 8 partial sums into the final scalar mean.

Host staging: slices per-core shards, transposes dets/heat to [h, p, w] so
every DMA descriptor is a 4.3KB contiguous run, and gathers the 510 predicted
tags per image (index staging for the tag loss).

Per-core device pipeline (per image b, stacks s=0..3):
  DVE   : diff = dets[b,s] - heat[b]      (layout [h=128, (p,w)=2176], bf16 2x)
  ACT/DVE: sq  = diff^2 into sq_img[:, s*2176:...] as bf16 (ACT Square runs
          1x, DVE bf16 mul 2x; one DVE square per image balances the engines
          against the DMA pace)
  PE    : 32 matmuls per image, each covering a group of 4 w-columns:
          lhsT = mask[:, 4g:4g+4] (bf16), rhs = sq viewed as [h, w4, s, p].
          Group g targets PE column quadrant g%4 (tile_position) so
          consecutive LDWEIGHTS+MATMUL pairs pipeline in different
          sub-arrays.  One PSUM region accumulates across ALL images
          (the loss sums over b anyway): useful results sit at
          psum[32q+j, 68j:68j+68] = sum over {w : w%16 == 4q+j} of m * sq.
  tag   : gathered pred tags packed [128,64]; (pt-gt)^2*vis summed on DVE in
          an early bubble (input arrives on the scalar HWDGE ring).

DMA plan: 20 single [128, 2176]-shaped transfers on the sync HWDGE ring in
need-order, one tile per transfer (multiple outstanding DMAs into one tile
serialize on its semaphore).  mask/tagin and the tag output ride the scalar
HWDGE ring; descriptors within a ring drain FIFO.
"""

import sys
import types

import ml_dtypes
import numpy as np

import concourse.bacc as bacc
import concourse.mybir as mybir
from concourse.tile import TileContext
from concourse.bass_utils import run_bass_kernel_spmd

# If BASS_TRACE is set in the environment but this image lacks
# antenv.axon_hooks, run_bass_kernel_spmd would die on import; register a
# no-op hook module so tracing degrades gracefully instead.
try:
    import antenv.axon_hooks  # noqa: F401
except ImportError:
    try:
        import antenv

        _m = types.ModuleType("antenv.axon_hooks")
        _m.get_axon_ntff_profile_hook = lambda: None
        _m.set_axon_ntff_profile_hook = lambda h: None
        sys.modules["antenv.axon_hooks"] = _m
        antenv.axon_hooks = _m
    except ImportError:
        pass

# Problem constants (hardcoded per harness contract)
B, S, C, H, W = 32, 4, 34, 128, 128
N_PARTS, TAG_DIM, M = 17, 1, 30
TAG_W, HM_W = 0.001, 1.0
NCORES = 8
BLOC = B // NCORES            # 4 images per core
FREE = N_PARTS * W            # 2176 free elems per (b, s) tile
KP = M * N_PARTS              # 510 keypoints per image
KP_COLS = 4                   # ceil(510 / 128) columns per (b, s)
TAG_COLS = BLOC * S * KP_COLS  # 64
WG = 4                        # w-columns per matmul group
NG = W // WG                  # 32 matmul groups per image
SQF = WG * S * N_PARTS        # 272 psum cols per image (w4, s, p)

_cache = {}

# DMA dets from HBM as fp8e4m3 and upcast to bf16 in-flight (SWDGE cast
# path).  Measured SLOWER than bf16 HWDGE: the stream is SBUF-write-side
# bound (11.1MB bf16 lands in SBUF either way) and the cast path costs more
# DMA-engine time per byte (28us vs 26us busy), so keep plain bf16.
FP8_DETS = False


def _build():
    f32 = mybir.dt.float32
    bf16 = mybir.dt.bfloat16
    det_dt = mybir.dt.float8e4 if FP8_DETS else mybir.dt.bfloat16
    nc = bacc.Bacc(
        "TRN2", target_bir_lowering=False, debug=False, num_devices=NCORES
    )
    # Host pre-interleaves dets into 2-stack pairs [pair, H, (s p w)] and
    # heat into 2-image pairs [pair, H, (b p w)] so every bulk transfer is a
    # single contiguous 8.7KB run per partition (max DMA-engine efficiency);
    # image 3's last two stacks stay as single-stack transfers [H, (p w)]
    # for fine-grained tail chasing.
    dets_p = nc.dram_tensor(
        "dets_p", [7, H, 2 * FREE], det_dt, kind="ExternalInput"
    )
    dets_s = nc.dram_tensor("dets_s", [2, H, FREE], det_dt, kind="ExternalInput")
    heat_p = nc.dram_tensor("heat_p", [2, H, 2 * FREE], bf16, kind="ExternalInput")
    maskw = nc.dram_tensor("maskw", [H, BLOC * W], bf16, kind="ExternalInput")
    tagin = nc.dram_tensor("tagin", [128, 3 * TAG_COLS], f32, kind="ExternalInput")
    out_det = nc.dram_tensor("out_det", [128, SQF], f32, kind="ExternalOutput")
    out_tag = nc.dram_tensor("out_tag", [128, 1], f32, kind="ExternalOutput")

    with TileContext(nc) as tc:
        with (
            tc.tile_pool(name="const", bufs=1) as cpool,
            tc.tile_pool(name="heatp", bufs=2) as hpool,
            tc.tile_pool(name="detp", bufs=7) as dpool,
            tc.tile_pool(name="dets3", bufs=2) as spool,
            tc.tile_pool(name="diffp", bufs=4) as fpool,
            tc.tile_pool(name="sqp", bufs=3) as qpool,
            tc.tile_pool(name="psum", bufs=1, space="PSUM") as ppool,
        ):
            # Small inputs ride the scalar HWDGE ring so they don't queue
            # behind the bulk det stream on the sync ring.
            mask_t = cpool.tile([128, BLOC * W], bf16)
            nc.scalar.dma_start(out=mask_t[:], in_=maskw[:])
            tag_t = cpool.tile([128, 3 * TAG_COLS], f32)
            nc.scalar.dma_start(out=tag_t[:], in_=tagin[:])

            # Dummy activation so the Square table set loads during the DMA
            # ramp instead of delaying the first real square.
            warm_t = cpool.tile([1, 8], f32)
            warm_o = cpool.tile([1, 8], f32)
            nc.gpsimd.memset(warm_t[:], 0.0)
            nc.scalar.activation(
                warm_o[:], warm_t[:], mybir.ActivationFunctionType.Square
            )

            # Bulk het/det stream on the sync ring: 20 single transfers, each
            # into its OWN tile.  Measured facts behind this shape: (a) the
            # [128, 2176] single-stack transfer with one 4.3KB run per
            # partition is the most DMA-engine-efficient unit (pairing
            # stacks into [128, 2, 2176] transfers measured ~10% slower per
            # byte); (b) multiple outstanding DMAs into ONE tile serialize
            # on that tile's semaphore, so every transfer gets its own tile;
            # (c) the dynamic HWDGE path keeps ~10 transfers in flight and
            # stalls the issuing sequencer until ~28us when full — with this
            # order the stalled issues are images 2-3, which still arrive
            # before they're needed.  Within the ring, descriptors drain
            # FIFO (emission order = arrival order).
            # 11 transfers up-front on the sync ring in need-order, one tile
            # per transfer.  Measured facts behind this shape: (a) single
            # contiguous runs per partition are the efficient DMA unit (the
            # host interleave makes pairs one 8.7KB run); (b) multiple
            # outstanding DMAs into ONE tile serialize on that tile's
            # semaphore; (c) the dynamic-HWDGE path keeps ~10 transfers in
            # flight and stalls the issuing sequencer until ~28us when full
            # — with 11 transfers only the last (needed ~41us) stalls, and
            # its descriptors still enter the ring in time.  Within a ring,
            # descriptors drain FIFO (emission order = arrival order).
            heat_tiles = {}
            det_tiles = {}
            pair_idx = 0
            for b0 in (0, 2):
                hp = hpool.tile([128, 2 * FREE], bf16, name="heat_t", tag="heat_t")
                nc.sync.dma_start(out=hp[:], in_=heat_p[b0 // 2])
                heat_tiles[b0] = hp[:, 0:FREE]
                heat_tiles[b0 + 1] = hp[:, FREE : 2 * FREE]
                for b in (b0, b0 + 1):
                    for s0 in (0, 2):
                        if b == BLOC - 1 and s0 == 2:
                            for s in (2, 3):
                                ds = spool.tile(
                                    [128, FREE], bf16, name="det_s", tag="det_s"
                                )
                                nc.sync.dma_start(out=ds[:], in_=dets_s[s - 2])
                                det_tiles[(b, s)] = ds[:]
                        else:
                            dp = dpool.tile(
                                [128, 2 * FREE], bf16, name="det_t", tag="det_t"
                            )
                            nc.sync.dma_start(out=dp[:], in_=dets_p[pair_idx])
                            pair_idx += 1
                            det_tiles[(b, s0)] = dp[:, 0:FREE]
                            det_tiles[(b, s0 + 1)] = dp[:, FREE : 2 * FREE]

            acc_det = cpool.tile([128, SQF], f32)
            # One PSUM region accumulates across ALL images (the final loss
            # sums over b anyway), so there is a single PSUM->SBUF copy and
            # a tiny output DMA at the very end.
            psum_t = ppool.tile([128, SQF], f32)
            psum_r = psum_t[:].rearrange("m (j s p) -> m j s p", j=WG, s=S)

            # ---- heatmap (det) loss ----
            for b in range(BLOC):
                if b == 1:
                    # Tag loss (tiny).  Emitted here so it sits behind image
                    # 0's subs in the DVE program: the first det sub is the
                    # critical op at pipeline start, the tag block fills a
                    # later DVE bubble.
                    ptg_t = tag_t[:, 0:TAG_COLS]
                    gtv_t = tag_t[:, TAG_COLS : 2 * TAG_COLS]
                    vis_t = tag_t[:, 2 * TAG_COLS : 3 * TAG_COLS]
                    e_t = cpool.tile([128, TAG_COLS], f32)
                    ev_t = cpool.tile([128, TAG_COLS], f32)
                    scr_t = cpool.tile([128, TAG_COLS], f32)
                    tag_acc = cpool.tile([128, 1], f32)
                    nc.vector.tensor_sub(e_t[:], ptg_t, gtv_t)
                    nc.vector.tensor_mul(ev_t[:], e_t[:], vis_t)
                    nc.vector.tensor_mul(scr_t[:], e_t[:], ev_t[:])
                    nc.vector.reduce_sum(
                        tag_acc[:], scr_t[:], axis=mybir.AxisListType.X
                    )
                    nc.scalar.dma_start(out=out_tag[:], in_=tag_acc[:])

                heat_t = heat_tiles[b]
                sq_t = qpool.tile([128, S * FREE], bf16)
                for s in range(S):
                    det_t = det_tiles[(b, s)]
                    diff_t = fpool.tile([128, FREE], bf16, name="diff_t", tag="diff_t")
                    dst = sq_t[:, s * FREE : (s + 1) * FREE]
                    # ACT squares run at 1x (2.1us); DVE bf16 muls at 2x
                    # (1.2us).  One DVE square per image balances the
                    # engines against the DMA pace; the final tile is
                    # processed in halves, fully on DVE, to shorten the tail.
                    nc.vector.tensor_sub(diff_t[:], det_t, heat_t)
                    # DVE paces the tail: keep its square load on the late
                    # images only (ACT has mid-stream slack for the rest).
                    if (s == 1 and b >= 2) or (b == BLOC - 1 and s == S - 1):
                        nc.vector.tensor_mul(dst, diff_t[:], diff_t[:])
                    else:
                        nc.scalar.activation(
                            dst, diff_t[:], mybir.ActivationFunctionType.Square
                        )

                # 32 matmuls per image, each covering 4 w-columns: lhsT =
                # mask 4-col slice, rhs = sq viewed [h, w4, s, p].  Group g
                # targets PE column quadrant g % 4 so consecutive
                # LDWEIGHTS+MATMUL pairs run concurrently in different
                # sub-arrays instead of serializing on the array-column
                # conflict.  Useful outputs are psum[32q+j, 68j:68(j+1)].
                # The last image's data all arrives at the stream's end, so
                # its rounds can't overlap the stream; two rounds ({s0,s1}
                # runs while DVE finishes s2/s3, then {s2,s3}) minimize
                # round overhead + inter-round semaphore waits in the tail.
                sq_g = sq_t[:].rearrange("q (s p w) -> q w s p", s=S, p=N_PARTS)
                rounds = [(0, S)] if b < BLOC - 1 else [(0, 2), (2, 4)]
                last_round = len(rounds) - 1
                for ri, (s0, s1) in enumerate(rounds):
                    for g in range(NG):
                        q = g % 4
                        nc.tensor.matmul(
                            psum_r[32 * q : 32 * q + 4, :, s0:s1, :],
                            lhsT=mask_t[:, b * W + WG * g : b * W + WG * (g + 1)],
                            rhs=sq_g[:, WG * g : WG * (g + 1), s0:s1, :],
                            start=(b == 0 and g < 4),
                            stop=(b == BLOC - 1 and ri == last_round and g >= NG - 4),
                            tile_position=(0, 32 * q),
                        )

            # Output DMA issues from the sync sequencer, which is parked on
            # its wait by then and fires in parallel with the copy (issuing
            # from ACT itself measured slower: the D2D issue serializes
            # behind the copy on the ACT queue).
            nc.scalar.copy(acc_det[:], psum_t[:])
            nc.sync.dma_start(out=out_det[:], in_=acc_det[:])
    nc.compile()
    return nc


def _pack(vals):
    """vals: (BLOC, S, KP) float32 -> [128, TAG_COLS] with col = b*16+s*4+j,
    partition k holding element j*128+k of the zero-padded 512 vector."""
    padded = np.zeros((BLOC, S, KP_COLS * 128), np.float32)
    padded[..., :KP] = vals
    return (
        padded.reshape(BLOC, S, KP_COLS, 128)
        .transpose(3, 0, 1, 2)
        .reshape(128, TAG_COLS)
    )


def kernel(preds, masks, keypoints_idx, keypoints_vis, gt_tags, heatmaps):
    preds = np.asarray(preds, dtype=np.float32)
    masks = np.asarray(masks, dtype=np.float32)
    keypoints_idx = np.asarray(keypoints_idx)
    keypoints_vis = np.asarray(keypoints_vis, dtype=np.float32)
    gt_tags = np.asarray(gt_tags, dtype=np.float32)
    heatmaps = np.asarray(heatmaps, dtype=np.float32)

    if "nc" not in _cache:
        _cache["nc"] = _build()
    nc = _cache["nc"]

    # Host-side input staging: gather predicted tags at keypoint locations
    # (index-based staging; all loss arithmetic runs on device).
    tags = preds[:, :, N_PARTS:].reshape(B, S, N_PARTS * H * W)
    flat_idx = keypoints_idx.reshape(B, 1, KP).astype(np.int64)
    pt = np.take_along_axis(tags, np.broadcast_to(flat_idx, (B, S, KP)), axis=2)
    gt = gt_tags.reshape(B, KP)
    vi = keypoints_vis.reshape(B, KP)

    in_maps = []
    for c in range(NCORES):
        b0 = c * BLOC
        sl = slice(b0, b0 + BLOC)
        tag_in = np.concatenate(
            [
                _pack(pt[sl]),
                _pack(np.broadcast_to(gt[sl][:, None, :], (BLOC, S, KP))),
                _pack(np.broadcast_to(vi[sl][:, None, :], (BLOC, S, KP))),
            ],
            axis=1,
        )
        det_np = ml_dtypes.float8_e4m3 if FP8_DETS else ml_dtypes.bfloat16
        # [BLOC, S, 17, H, W] -> [BLOC, S, H, 17, W], then pack 2-stack
        # pairs as [H, (s p w)] and heat 2-image pairs as [H, (b p w)] so
        # each bulk DMA is one contiguous run per partition.
        dt_ = preds[sl, :, :N_PARTS].transpose(0, 1, 3, 2, 4)
        ht_ = heatmaps[sl].transpose(0, 2, 1, 3)
        dets_p = np.stack(
            [
                dt_[b, s0 : s0 + 2].transpose(1, 0, 2, 3).reshape(H, 2 * FREE)
                for b, s0 in (
                    (0, 0), (0, 2), (1, 0), (1, 2), (2, 0), (2, 2), (3, 0),
                )
            ]
        )
        dets_s = dt_[3, 2:4].reshape(2, H, FREE)
        heat_pk = np.stack(
            [
                ht_[b0 : b0 + 2].transpose(1, 0, 2, 3).reshape(H, 2 * FREE)
                for b0 in (0, 2)
            ]
        )
        in_maps.append(
            {
                "dets_p": np.ascontiguousarray(dets_p).astype(det_np),
                "dets_s": np.ascontiguousarray(dets_s).astype(det_np),
                "heat_p": np.ascontiguousarray(heat_pk).astype(ml_dtypes.bfloat16),
                # [BLOC, H, W] -> [H, BLOC*W]
                "maskw": np.ascontiguousarray(
                    masks[sl].transpose(1, 0, 2).reshape(H, BLOC * W)
                ).astype(ml_dtypes.bfloat16),
                "tagin": np.ascontiguousarray(tag_in),
            }
        )

    res = run_bass_kernel_spmd(nc, in_maps, list(range(NCORES)))
    _cache["last_results"] = res

    det_total = 0.0
    tag_total = 0.0
    for r in res.results:
        od = r["out_det"].astype(np.float64).reshape(4, 32, WG, S * N_PARTS)
        # row 32q+i, diag block i holds the w % 16 == 4q+i partials
        for q in range(4):
            for i in range(WG):
                det_total += float(od[q, i, i, :].sum())
        tag_total += float(r["out_tag"].sum(dtype=np.float64))

    det_mean = det_total / (B * S * N_PARTS * H * W)
    tag_mean = tag_total / (B * S)
    return np.float32(TAG_W * tag_mean + HM_W * det_mean)
